# revision 19
# baseline (speedup 1.0000x reference)
"""Trainium2 Bass kernel for CIN layer:
    out[b,c,d] = sum_{h,m} W[c, h*M+m] * xk[b,h,d] * x0[b,m,d] + bias[c]

Shapes (hardcoded): x0 [512,40,64] f32, xk [512,128,64] f32,
W [128,5120] f32, b [128] f32 -> out [512,128,64] f32.

Strategy: data-parallel over batch B across 8 cores (64 batches/core).
Per core, columns are the 64*64=4096 (b,d) pairs. The 5120-long (h,m)
contraction is split into 40 chunks of 128 rows with a mixed-radix
partition layout: chunk (g, j) covers m in the 8-wide group g (5 groups)
x h in the 16-wide block j (8 blocks); partition p holds
(m = 8g + p//16, h = 16j + p%16). Then
  outer[p, col] = xkrep_j[p, col] * x0bc_g[p, col]  (elementwise, bf16)
  psum[q] += w3[k][p,c].T @ outer[:, q*512:...]     (PE, accum 40 chunks)
xkrep_j / x0bc_g are produced host-side (pure layout, no arithmetic).

The elementwise outer-product stream (21M elem/core) exceeds what the
DVE alone can sustain (bf16 tensor_tensor caps at 2 elem/lane/cycle),
so the last chunk of each 5-chunk group is produced on GpSimd instead.
GpSimd chunks are consumed one group LATE (lag-1) so the slower engine
(~4.2us/chunk vs DVE ~1.1us) is never on the PE's critical path.

Columns are processed in two phases of 2048 so the PSUM eviction (bias
add on ScalarE + store) of phase 0 overlaps phase 1 compute.  The first
group's DVE chunks and their input DMAs are split into 1024-col quarters
so the first matmul can start ~2.5us in instead of waiting for full
half-tiles.  W is stored pre-transposed [128, 40*128] and the output
DRAM tensor is c-major [C, BC, D] so every DMA descriptor moves >=2KB
contiguous (sub-512B descriptors pay a 2x DMA-time penalty).
"""

import numpy as np
import ml_dtypes

B, M, H, D, C = 512, 40, 128, 64, 128
N_CORES = 8
BC = B // N_CORES          # 64 batches per core
COLS = BC * D              # 4096 (b,d) columns per core
NG = 8                     # PSUM groups
GW = COLS // NG            # 512 columns per group
MG = 8                     # m-values per chunk group
NMG = M // MG              # 5 m-groups
HB = 128 // MG             # 16 h-values per block
NHB = H // HB              # 8 h-blocks
NCHUNK = NMG * NHB         # 40 contraction chunks

_cache = {}


def _build(reps=1):
    import contextlib

    import concourse.bacc as bacc
    import concourse.mybir as mybir
    from concourse.tile import TileContext

    f32 = mybir.dt.float32
    bf16 = mybir.dt.bfloat16

    nc = bacc.Bacc("TRN2", debug=False, num_devices=N_CORES)

    xkr_d = nc.dram_tensor("xkrep_in", [NHB, 128, COLS], bf16, kind="ExternalInput")
    x0b_d = nc.dram_tensor("x0bc_in", [NMG, 128, COLS], bf16, kind="ExternalInput")
    # pre-transposed: partition-major, 10KB contiguous per partition row
    w3_d = nc.dram_tensor("w3_in", [128, NCHUNK * C], bf16, kind="ExternalInput")
    bias_d = nc.dram_tensor("bias_in", [C, 1], f32, kind="ExternalInput")
    # c-major so each output descriptor is a contiguous 2KB (b,d) run
    out_d = nc.dram_tensor("out", [C, BC, D], f32, kind="ExternalOutput")

    GK = 5
    NGRP = NCHUNK // GK        # 8 groups per phase
    ND_SLOT = 9
    NG_SLOT = 4
    HCOL = COLS // 2
    QCOL = COLS // 4

    with TileContext(nc) as tc:
        with (
            tc.tile_pool(name="const", bufs=1) as cpool,
            tc.tile_pool(name="work", bufs=6) as wpool,
            tc.tile_pool(name="outp", bufs=1) as opool,
            tc.tile_pool(name="psum", bufs=1, space="PSUM") as ppool,
        ):
            # ---- SBUF constant tiles ----
            w3_sb = cpool.tile([128, NCHUNK * C], bf16)
            bias_sb = cpool.tile([128, 1], f32)
            xkreps = [
                cpool.tile([128, COLS], bf16, name=f"xkr{i}", tag=f"xkr{i}")
                for i in range(NHB)
            ]
            x0bcs = [
                cpool.tile([128, COLS], bf16, name=f"x0b{i}", tag=f"x0b{i}")
                for i in range(NMG)
            ]

            # ---- prologue DMA, first-use order ----
            # Group-0 operand tiles at quarter-column granularity (with the
            # w3 chunk-0..9 slice third) so the first TT/MM chain starts
            # ~2.5us in; then everything else in first-use order.
            def load_tile(kind, i, c0, c1):
                t = xkreps[i] if kind == "x" else x0bcs[i]
                src = (xkr_d if kind == "x" else x0b_d).ap()[i]
                nc.sync.dma_start(out=t[:, c0:c1], in_=src[:, c0:c1])

            use_order = [("x", 0), ("0", 0), ("x", 1), ("x", 2), ("x", 3),
                         ("x", 4), ("x", 5), ("x", 6), ("x", 7),
                         ("0", 1), ("0", 2), ("0", 3), ("0", 4)]
            # bias first (512B, and every eviction FIFO-waits on it), then
            # pass-A quarters with the w3 thirds interleaved, then pass-B
            # quarters, then pass-C halves - each batch in first-use order
            nc.sync.dma_start(out=bias_sb, in_=bias_d.ap())
            for n, (kind, i) in enumerate(use_order):
                load_tile(kind, i, 0, QCOL)
                if n == 1:
                    nc.sync.dma_start(
                        out=w3_sb[:, : 10 * C], in_=w3_d.ap()[:, : 10 * C]
                    )
                elif n == 4:
                    nc.sync.dma_start(
                        out=w3_sb[:, 10 * C: 20 * C],
                        in_=w3_d.ap()[:, 10 * C: 20 * C],
                    )
                elif n == 6:
                    nc.sync.dma_start(
                        out=w3_sb[:, 20 * C:], in_=w3_d.ap()[:, 20 * C:]
                    )
            for kind, i in use_order:
                load_tile(kind, i, QCOL, HCOL)
            for kind, i in use_order:
                load_tile(kind, i, HCOL, COLS)

            loop_ctx = (
                tc.For_i(
                    0, reps, 1,
                    hint_engines=(mybir.EngineType.PE,),
                    staggered_reset=True,
                )
                if reps > 1
                else contextlib.nullcontext()
            )
            with loop_ctx:
                psums = []
                for q in range(NG):
                    ps = ppool.tile([128, GW], f32, name=f"ps{q}", tag=f"ps{q}")
                    psums.append(ps)

                if reps == 1:
                    # Warm the PE (HAM clock-gate needs ~3.4us of sustained
                    # activity to reach 2.4 GHz) with dummy matmuls while
                    # the first operand DMAs are in flight.  start=True on
                    # each real first-accumulation MM discards the garbage.
                    scratch = cpool.tile([128, GW], bf16)
                    nc.gpsimd.memset(scratch, 0.0)
                    for _ in range(12):
                        nc.tensor.matmul(
                            psums[0],
                            lhsT=scratch[:, :128],
                            rhs=scratch,
                            start=True,
                            stop=True,
                        )

                # ---- main loop: three column passes over 8 chunk groups --
                # Pass A covers cols [0,1024) into PSUM banks 0-1, pass B
                # [1024,2048) into banks 2-3, pass C [2048,4096) into banks
                # 4-7.  The narrow bootstrap passes keep the PE fed while
                # the prologue DMA stream is still arriving; pass C uses
                # full half-width ops once everything is resident.
                # GpSimd-produced chunks are consumed `lag` groups after
                # production so the slow engine never gates the PE; pass
                # A/B picks respect prologue DMA arrival times.
                # sched = {chunk: cons_group}
                gp_a = {1: 2, 5: 3, 12: 4, 18: 5, 21: 6, 26: 7, 31: 7}
                gp_c = {1: 1, 4: 2, 9: 3, 14: 4, 19: 5, 24: 6, 29: 7, 34: 7}
                passes = [
                    ("A", 0, QCOL, [0, 1], gp_a),
                    ("B", QCOL, 2 * QCOL, [2, 3], gp_a),
                    ("C", 2 * QCOL, 4 * QCOL, [4, 5, 6, 7], gp_c),
                ]
                bpg = BC // NG
                for pname, c0, c1, banks, sched in passes:
                    width = c1 - c0
                    nq = width // GW
                    ndve = 0
                    ngp = 0
                    pending = {}    # cons_group -> [(k, tile)]
                    for gi in range(NGRP):
                        k0 = gi * GK
                        last_grp = gi == NGRP - 1
                        mm_list = list(pending.pop(gi, []))
                        for k in range(k0, k0 + GK):
                            g, j = divmod(k, NHB)
                            if k in sched:
                                eng, tag, lst = (
                                    nc.gpsimd, f"og{pname in 'AB'}"
                                    f"{ngp % NG_SLOT}", None,
                                )
                                outer = wpool.tile(
                                    [128, width], bf16,
                                    name=f"outg{pname}_{k}", tag=tag, bufs=1,
                                )
                                ngp += 1
                                nc.gpsimd.tensor_mul(
                                    outer,
                                    xkreps[j][:, c0:c1],
                                    x0bcs[g][:, c0:c1],
                                )
                                pending.setdefault(sched[k], []).append(
                                    (k, outer)
                                )
                                continue
                            outer = wpool.tile(
                                [128, width], bf16, name=f"outd{pname}_{k}",
                                tag=f"od{pname in 'AB'}{ndve % (ND_SLOT if pname in 'AB' else ND_SLOT - 1)}",
                                bufs=1,
                            )
                            ndve += 1
                            nc.vector.tensor_mul(
                                outer,
                                xkreps[j][:, c0:c1],
                                x0bcs[g][:, c0:c1],
                            )
                            mm_list.append((k, outer))
                        for qi, qb in enumerate(banks):
                            for n, (k, outer) in enumerate(mm_list):
                                nc.tensor.matmul(
                                    psums[qb],
                                    lhsT=w3_sb[:, k * C:(k + 1) * C],
                                    rhs=outer[:, qi * GW:(qi + 1) * GW],
                                    start=(k == 0),
                                    stop=(last_grp and n == len(mm_list) - 1),
                                )
                    assert not pending
                    # bias add into a shared SBUF tile; contiguous multi-bank
                    # stores (>=2KB per-partition descriptors).  The final
                    # pass stores 3+1 banks so the post-last-matmul chain is
                    # only one bank's evict + a 512-col store.
                    out_sb = opool.tile(
                        [128, width], f32, name=f"osb{pname}", tag=f"osb{pname}"
                    )
                    for qi, qb in enumerate(banks):
                        nc.scalar.activation(
                            out_sb[:, qi * GW:(qi + 1) * GW],
                            psums[qb],
                            mybir.ActivationFunctionType.Identity,
                            bias=bias_sb[:, 0:1],
                            scale=1.0,
                        )
                    # final pass: single-bank stores so the post-last-matmul
                    # chain is evict + one 728ns store, with earlier banks'
                    # stores pipelining ahead on the DMA engines
                    stores = [banks] if len(banks) == 2 else [[b] for b in banks]
                    for sb in stores:
                        lo = sb[0] * bpg
                        hi = (sb[-1] + 1) * bpg
                        nc.sync.dma_start(
                            out=out_d.ap()[:, lo:hi, :],
                            in_=out_sb[:, sb[0] * GW - banks[0] * GW:
                                       (sb[-1] + 1) * GW - banks[0] * GW],
                        )

    nc.compile()
    return nc


def _prep_host(x0, xk, W, b):
    """Host-side layout prep (no arithmetic): shard, transpose, replicate."""
    part = np.arange(128)
    hh = (part % HB)[None, :] + HB * np.arange(NHB)[:, None]   # [NHB, 128]
    mm = (part // HB)[None, :] + MG * np.arange(NMG)[:, None]  # [NMG, 128]

    Wr = W.reshape(C, H, M)
    w3 = np.empty((128, NCHUNK, C), ml_dtypes.bfloat16)
    for g in range(NMG):
        for j in range(NHB):
            w3[:, g * NHB + j, :] = Wr[:, hh[j], mm[g]].T.astype(
                ml_dtypes.bfloat16
            )
    w3 = np.ascontiguousarray(w3.reshape(128, NCHUNK * C))
    bias = np.ascontiguousarray(b.reshape(C, 1)).astype(np.float32)

    in_maps = []
    for k in range(N_CORES):
        x0s = x0[k * BC:(k + 1) * BC]            # [BC, M, D]
        xks = xk[k * BC:(k + 1) * BC]            # [BC, H, D]
        xk2 = (
            np.ascontiguousarray(xks.transpose(1, 0, 2))
            .reshape(H, COLS)
            .astype(ml_dtypes.bfloat16)
        )
        x02 = (
            np.ascontiguousarray(x0s.transpose(1, 0, 2))
            .reshape(M, COLS)
            .astype(ml_dtypes.bfloat16)
        )
        in_maps.append(
            {
                "xkrep_in": np.ascontiguousarray(xk2[hh]),
                "x0bc_in": np.ascontiguousarray(x02[mm]),
                "w3_in": w3,
                "bias_in": bias,
            }
        )
    return in_maps


def _run(in_maps, **kwargs):
    from concourse import bass_utils

    if "nc" not in _cache:
        _cache["nc"] = _build()
    return bass_utils.run_bass_kernel_spmd(
        _cache["nc"], in_maps, core_ids=list(range(N_CORES)), **kwargs
    )


def kernel(x0, xk, W, b, _bench=[None]):
    x0 = np.asarray(x0, dtype=np.float32)
    xk = np.asarray(xk, dtype=np.float32)
    W = np.asarray(W, dtype=np.float32)
    b = np.asarray(b, dtype=np.float32)
    in_maps = _prep_host(x0, xk, W, b)
    res = _run(in_maps)
    _bench[0] = res
    # per-core out is c-major [C, BC, D]; restore [BC, C, D] and stack cores
    out = np.concatenate(
        [np.transpose(r["out"], (1, 0, 2)) for r in res.results], axis=0
    )
    return np.ascontiguousarray(out, dtype=np.float32)


# revision 22
# speedup vs baseline: 1.7356x; 1.7356x over previous
"""Trainium2 Bass kernel for CIN layer:
    out[b,c,d] = sum_{h,m} W[c, h*M+m] * xk[b,h,d] * x0[b,m,d] + bias[c]

Shapes (hardcoded): x0 [512,40,64] f32, xk [512,128,64] f32,
W [128,5120] f32, b [128] f32 -> out [512,128,64] f32.

Strategy: data-parallel over batch B across 8 cores (64 batches/core).
Per core, columns are the 64*64=4096 (b,d) pairs. The 5120-long (h,m)
contraction is split into 40 chunks of 128 rows with a mixed-radix
partition layout: chunk (g, j) covers m in the 8-wide group g (5 groups)
x h in the 16-wide block j (8 blocks); partition p holds
(m = 8g + p//16, h = 16j + p%16). Then
  outer[p, col] = xkrep_j[p, col] * x0bc_g[p, col]  (elementwise, bf16)
  psum[q] += w3[k][p,c].T @ outer[:, q*512:...]     (PE, accum 40 chunks)
xkrep_j / x0bc_g are produced host-side (pure layout, no arithmetic).

The elementwise outer-product stream (21M elem/core) exceeds what the
DVE alone can sustain (bf16 tensor_tensor caps at 2 elem/lane/cycle),
so the last chunk of each 5-chunk group is produced on GpSimd instead.
GpSimd chunks are consumed one group LATE (lag-1) so the slower engine
(~4.2us/chunk vs DVE ~1.1us) is never on the PE's critical path.

Columns are processed in two phases of 2048 so the PSUM eviction (bias
add on ScalarE + store) of phase 0 overlaps phase 1 compute.  The first
group's DVE chunks and their input DMAs are split into 1024-col quarters
so the first matmul can start ~2.5us in instead of waiting for full
half-tiles.  W is stored pre-transposed [128, 40*128] and the output
DRAM tensor is c-major [C, BC, D] so every DMA descriptor moves >=2KB
contiguous (sub-512B descriptors pay a 2x DMA-time penalty).
"""

import numpy as np
import ml_dtypes

B, M, H, D, C = 512, 40, 128, 64, 128
N_CORES = 8
BC = B // N_CORES          # 64 batches per core
COLS = BC * D              # 4096 (b,d) columns per core
NG = 8                     # PSUM groups
GW = COLS // NG            # 512 columns per group
MG = 8                     # m-values per chunk group
NMG = M // MG              # 5 m-groups
HB = 128 // MG             # 16 h-values per block
NHB = H // HB              # 8 h-blocks
NCHUNK = NMG * NHB         # 40 contraction chunks

_cache = {}


def _build(reps=1):
    import contextlib

    import concourse.bacc as bacc
    import concourse.mybir as mybir
    from concourse.tile import TileContext

    f32 = mybir.dt.float32
    bf16 = mybir.dt.bfloat16

    nc = bacc.Bacc("TRN2", debug=False, num_devices=N_CORES)

    xkr_d = nc.dram_tensor("xkrep_in", [NHB, 128, COLS], bf16, kind="ExternalInput")
    x0b_d = nc.dram_tensor("x0bc_in", [NMG, 128, COLS], bf16, kind="ExternalInput")
    # pre-transposed: partition-major, 10KB contiguous per partition row
    w3_d = nc.dram_tensor("w3_in", [128, NCHUNK * C], bf16, kind="ExternalInput")
    bias_d = nc.dram_tensor("bias_in", [C, 1], f32, kind="ExternalInput")
    # c-major so each output descriptor is a contiguous 2KB (b,d) run
    out_d = nc.dram_tensor("out", [C, BC, D], f32, kind="ExternalOutput")

    GK = 5
    NGRP = NCHUNK // GK        # 8 groups per phase
    ND_SLOT = 9
    NG_SLOT = 4
    HCOL = COLS // 2
    QCOL = COLS // 4

    with TileContext(nc) as tc:
        with (
            tc.tile_pool(name="const", bufs=1) as cpool,
            tc.tile_pool(name="work", bufs=6) as wpool,
            tc.tile_pool(name="outp", bufs=1) as opool,
            tc.tile_pool(name="psum", bufs=1, space="PSUM") as ppool,
        ):
            # ---- SBUF constant tiles ----
            w3_sb = cpool.tile([128, NCHUNK * C], bf16)
            bias_sb = cpool.tile([128, 1], f32)
            xkreps = [
                cpool.tile([128, COLS], bf16, name=f"xkr{i}", tag=f"xkr{i}")
                for i in range(NHB)
            ]
            x0bcs = [
                cpool.tile([128, COLS], bf16, name=f"x0b{i}", tag=f"x0b{i}")
                for i in range(NMG)
            ]

            # ---- prologue DMA, first-use order ----
            # Group-0 operand tiles at quarter-column granularity (with the
            # w3 chunk-0..9 slice third) so the first TT/MM chain starts
            # ~2.5us in; then everything else in first-use order.
            def load_tile(kind, i, c0, c1):
                t = xkreps[i] if kind == "x" else x0bcs[i]
                src = (xkr_d if kind == "x" else x0b_d).ap()[i]
                nc.sync.dma_start(out=t[:, c0:c1], in_=src[:, c0:c1])

            use_order = [("x", 0), ("0", 0), ("x", 1), ("x", 2), ("x", 3),
                         ("x", 4), ("x", 5), ("x", 6), ("x", 7),
                         ("0", 1), ("0", 2), ("0", 3), ("0", 4)]
            # bias first (512B, and every eviction FIFO-waits on it), then
            # pass-A quarters with the w3 thirds interleaved, then pass-B
            # quarters, then pass-C halves - each batch in first-use order
            nc.sync.dma_start(out=bias_sb, in_=bias_d.ap())
            for n, (kind, i) in enumerate(use_order):
                load_tile(kind, i, 0, QCOL)
                if n == 1:
                    nc.sync.dma_start(
                        out=w3_sb[:, : 10 * C], in_=w3_d.ap()[:, : 10 * C]
                    )
                elif n == 4:
                    nc.sync.dma_start(
                        out=w3_sb[:, 10 * C: 20 * C],
                        in_=w3_d.ap()[:, 10 * C: 20 * C],
                    )
                elif n == 6:
                    nc.sync.dma_start(
                        out=w3_sb[:, 20 * C:], in_=w3_d.ap()[:, 20 * C:]
                    )
            for kind, i in use_order:
                load_tile(kind, i, QCOL, HCOL)
            for kind, i in use_order:
                load_tile(kind, i, HCOL, COLS)

            loop_ctx = (
                tc.For_i(
                    0, reps, 1,
                    hint_engines=(mybir.EngineType.PE,),
                    staggered_reset=True,
                )
                if reps > 1
                else contextlib.nullcontext()
            )
            with loop_ctx:
                psums = []
                for q in range(NG):
                    ps = ppool.tile([128, GW], f32, name=f"ps{q}", tag=f"ps{q}")
                    psums.append(ps)

                if reps == 1:
                    # Warm the PE (HAM clock-gate needs ~3.4us of sustained
                    # activity to reach 2.4 GHz) with dummy matmuls while
                    # the first operand DMAs are in flight.  start=True on
                    # each real first-accumulation MM discards the garbage.
                    scratch = cpool.tile([128, GW], bf16)
                    nc.gpsimd.memset(scratch, 0.0)
                    for _ in range(12):
                        nc.tensor.matmul(
                            psums[0],
                            lhsT=scratch[:, :128],
                            rhs=scratch,
                            start=True,
                            stop=True,
                        )

                # ---- main loop: three column passes over 8 chunk groups --
                # Pass A covers cols [0,1024) into PSUM banks 0-1, pass B
                # [1024,2048) into banks 2-3, pass C [2048,4096) into banks
                # 4-7.  The narrow bootstrap passes keep the PE fed while
                # the prologue DMA stream is still arriving; pass C uses
                # full half-width ops once everything is resident.
                # GpSimd-produced chunks are consumed `lag` groups after
                # production so the slow engine never gates the PE; pass
                # A/B picks respect prologue DMA arrival times.
                # sched = {chunk: cons_group}
                gp_a = {1: 2, 5: 3, 12: 4, 18: 5, 21: 6, 26: 7, 31: 7}
                gp_c = {1: 1, 4: 2, 9: 3, 14: 4, 19: 5, 24: 6, 29: 7, 34: 7}
                if reps == 1:
                    # single-shot: narrow bootstrap passes overlap the
                    # prologue DMA stream
                    passes = [
                        ("A", 0, QCOL, [0, 1], gp_a, 9, 4),
                        ("B", QCOL, 2 * QCOL, [2, 3], gp_a, 9, 4),
                        ("C", 2 * QCOL, 4 * QCOL, [4, 5, 6, 7], gp_c, 8, 4),
                    ]
                else:
                    # steady-state loop: everything is resident; two
                    # half-width phases minimize per-iteration op count
                    passes = [
                        ("A", 0, HCOL, [0, 1, 2, 3], gp_c, 6, 3),
                        ("C", HCOL, 2 * HCOL, [4, 5, 6, 7], gp_c, 6, 3),
                    ]
                bpg = BC // NG
                for pname, c0, c1, banks, sched, n_od, n_og in passes:
                    width = c1 - c0
                    nq = width // GW
                    ndve = 0
                    ngp = 0
                    pending = {}    # cons_group -> [(k, tile)]
                    for gi in range(NGRP):
                        k0 = gi * GK
                        last_grp = gi == NGRP - 1
                        mm_list = list(pending.pop(gi, []))
                        for k in range(k0, k0 + GK):
                            g, j = divmod(k, NHB)
                            if k in sched:
                                outer = wpool.tile(
                                    [128, width], bf16,
                                    name=f"outg{pname}_{k}",
                                    tag=f"og{width}_{ngp % n_og}", bufs=1,
                                )
                                ngp += 1
                                nc.gpsimd.tensor_mul(
                                    outer,
                                    xkreps[j][:, c0:c1],
                                    x0bcs[g][:, c0:c1],
                                )
                                pending.setdefault(sched[k], []).append(
                                    (k, outer)
                                )
                                continue
                            outer = wpool.tile(
                                [128, width], bf16, name=f"outd{pname}_{k}",
                                tag=f"od{width}_{ndve % n_od}",
                                bufs=1,
                            )
                            ndve += 1
                            nc.vector.tensor_mul(
                                outer,
                                xkreps[j][:, c0:c1],
                                x0bcs[g][:, c0:c1],
                            )
                            mm_list.append((k, outer))
                        for qi, qb in enumerate(banks):
                            for n, (k, outer) in enumerate(mm_list):
                                nc.tensor.matmul(
                                    psums[qb],
                                    lhsT=w3_sb[:, k * C:(k + 1) * C],
                                    rhs=outer[:, qi * GW:(qi + 1) * GW],
                                    start=(k == 0),
                                    stop=(last_grp and n == len(mm_list) - 1),
                                )
                    assert not pending
                    # bias add into a shared SBUF tile; contiguous multi-bank
                    # stores (>=2KB per-partition descriptors).  The final
                    # pass stores 3+1 banks so the post-last-matmul chain is
                    # only one bank's evict + a 512-col store.
                    out_sb = opool.tile(
                        [128, width], f32, name=f"osb{pname}", tag=f"osb{pname}"
                    )
                    for qi, qb in enumerate(banks):
                        nc.scalar.activation(
                            out_sb[:, qi * GW:(qi + 1) * GW],
                            psums[qb],
                            mybir.ActivationFunctionType.Identity,
                            bias=bias_sb[:, 0:1],
                            scale=1.0,
                        )
                    # final pass: single-bank stores so the post-last-matmul
                    # chain is evict + one 728ns store, with earlier banks'
                    # stores pipelining ahead on the DMA engines
                    stores = [banks] if len(banks) == 2 else [[b] for b in banks]
                    for sb in stores:
                        lo = sb[0] * bpg
                        hi = (sb[-1] + 1) * bpg
                        nc.sync.dma_start(
                            out=out_d.ap()[:, lo:hi, :],
                            in_=out_sb[:, sb[0] * GW - banks[0] * GW:
                                       (sb[-1] + 1) * GW - banks[0] * GW],
                        )

    nc.compile()
    return nc


def _prep_host(x0, xk, W, b):
    """Host-side layout prep (no arithmetic): shard, transpose, replicate."""
    part = np.arange(128)
    hh = (part % HB)[None, :] + HB * np.arange(NHB)[:, None]   # [NHB, 128]
    mm = (part // HB)[None, :] + MG * np.arange(NMG)[:, None]  # [NMG, 128]

    Wr = W.reshape(C, H, M)
    w3 = np.empty((128, NCHUNK, C), ml_dtypes.bfloat16)
    for g in range(NMG):
        for j in range(NHB):
            w3[:, g * NHB + j, :] = Wr[:, hh[j], mm[g]].T.astype(
                ml_dtypes.bfloat16
            )
    w3 = np.ascontiguousarray(w3.reshape(128, NCHUNK * C))
    bias = np.ascontiguousarray(b.reshape(C, 1)).astype(np.float32)

    in_maps = []
    for k in range(N_CORES):
        x0s = x0[k * BC:(k + 1) * BC]            # [BC, M, D]
        xks = xk[k * BC:(k + 1) * BC]            # [BC, H, D]
        xk2 = (
            np.ascontiguousarray(xks.transpose(1, 0, 2))
            .reshape(H, COLS)
            .astype(ml_dtypes.bfloat16)
        )
        x02 = (
            np.ascontiguousarray(x0s.transpose(1, 0, 2))
            .reshape(M, COLS)
            .astype(ml_dtypes.bfloat16)
        )
        in_maps.append(
            {
                "xkrep_in": np.ascontiguousarray(xk2[hh]),
                "x0bc_in": np.ascontiguousarray(x02[mm]),
                "w3_in": w3,
                "bias_in": bias,
            }
        )
    return in_maps


def _run(in_maps, **kwargs):
    from concourse import bass_utils

    if "nc" not in _cache:
        _cache["nc"] = _build()
    return bass_utils.run_bass_kernel_spmd(
        _cache["nc"], in_maps, core_ids=list(range(N_CORES)), **kwargs
    )


def kernel(x0, xk, W, b, _bench=[None]):
    x0 = np.asarray(x0, dtype=np.float32)
    xk = np.asarray(xk, dtype=np.float32)
    W = np.asarray(W, dtype=np.float32)
    b = np.asarray(b, dtype=np.float32)
    in_maps = _prep_host(x0, xk, W, b)
    res = _run(in_maps)
    _bench[0] = res
    # per-core out is c-major [C, BC, D]; restore [BC, C, D] and stack cores
    out = np.concatenate(
        [np.transpose(r["out"], (1, 0, 2)) for r in res.results], axis=0
    )
    return np.ascontiguousarray(out, dtype=np.float32)
